# revision 26
# baseline (speedup 1.0000x reference)
import numpy as np

# Persistent XLA compilation cache: first-call jit compile of the NEFF
# custom-call wrapper is cached across processes.
try:
    import jax
    jax.config.update("jax_compilation_cache_dir", "/tmp/jax_comp_cache")
    jax.config.update("jax_persistent_cache_min_entry_size_bytes", -1)
    jax.config.update("jax_persistent_cache_min_compile_time_secs", 0.0)
except Exception:
    pass

# CNN-biLSTM-CRF forward NLL, data-parallel over batch across 8 NeuronCores.
#
# The axon transport costs ~80ms per blocking dispatch regardless of size, so
# the whole model runs in ONE cached jitted NEFF call with tiny per-call I/O:
#   - the 30MB f8 word table and all weights stay device-resident across calls
#   - per call we upload only token indices (4KB), char-CNN features (30KB)
#     and the mask-weighted gold-tag one-hot (25KB) per core
#   - the device gathers embeddings (indirect DMA), runs input projections,
#     the full biLSTM recurrence, the emission linear layer AND the CRF
#     forward algorithm (scaled/normalized-probability form), returning only
#     per-step log-normalizers + logZ candidates (4KB) and the gold-path
#     emission partial sums (0.8KB) per core
#   - host does embedding-free score terms and the length-dependent selects.

B, S, LW = 64, 128, 20
CHAR_E, CHAR_C = 30, 30
WORD_V, WORD_E = 100000, 300
H, NCLS = 256, 25
F = WORD_E + CHAR_C  # 330
KA = F + 1  # ones row folds the gate bias into the matmul
NCORES = 8
BC = B // NCORES  # 8 examples per core
R = BC * S  # 1024 rows per core, time-major: row = t*BC + e
NW = 8 * H  # 2048 = both directions' 4H gates
H4 = 4 * H

# gate order on device: i, f, o, g  (sigmoid on [0:3H), tanh on [3H:4H))
_GATE_PERM = np.concatenate([
    np.arange(0, H),            # i
    np.arange(H, 2 * H),        # f
    np.arange(3 * H, 4 * H),    # o
    np.arange(2 * H, 3 * H),    # g
])

# device-resident (cached across calls) vs per-call inputs
_WEIGHT_NAMES = ("wtab", "wsh", "whh_s", "linT", "expT", "cend", "estart",
                 "linb")


def _build_nc():
    import os
    import concourse.bacc as bacc
    import concourse.mybir as mybir
    from concourse import tile, bass
    from concourse.masks import make_identity

    bisect = os.environ.get("KBISECT", "")

    f32 = mybir.dt.float32
    bf16 = mybir.dt.bfloat16
    f8 = mybir.dt.float8e4
    i32 = mybir.dt.int32
    AF = mybir.ActivationFunctionType

    nc = bacc.Bacc("TRN2", target_bir_lowering=False, debug=False,
                   num_devices=NCORES)
    # --- resident inputs ---
    wtab = nc.dram_tensor("wtab", [WORD_V, WORD_E], f8, kind="ExternalInput")
    wsh = nc.dram_tensor("wsh", [KA, NW // 8], f8, kind="ExternalInput")
    whh_s = nc.dram_tensor("whh_s", [2 * H, H4 // 8], bf16,
                           kind="ExternalInput")
    linT = nc.dram_tensor("linT", [2 * H, NCLS], bf16, kind="ExternalInput")
    expT = nc.dram_tensor("expT", [NCLS, NCLS], f32, kind="ExternalInput")
    cend = nc.dram_tensor("cend", [NCLS, 2], f32, kind="ExternalInput")
    estart = nc.dram_tensor("estart", [NCLS, 1], f32, kind="ExternalInput")
    linb = nc.dram_tensor("linb", [1, NCLS], bf16, kind="ExternalInput")
    # --- per-call input, packed: every extra jit arg costs ~5-10ms on axon.
    # rows 0..3   = sidx  [128,8] int32 bytes (partition-major)
    # rows 4..33  = charT [30,1024] f8
    # rows 34..58 = wtag  [25,1024] f8
    u8 = mybir.dt.uint8
    pcall = nc.dram_tensor("pcall", [59, 1024], u8, kind="ExternalInput")
    # --- single output: one extra jit output costs a full axon round trip,
    # so the per-example gold-emission sums ride along as f32 bit-packed
    # into spare bf16 columns 1024:1040 of row 0 ---
    LZC = R + 16
    lz = nc.dram_tensor("lz", [2, LZC], bf16, kind="ExternalOutput")
    # collectives can't touch I/O tensors: bounce in, gather to Shared
    wT_b = nc.dram_tensor("wT_b", [KA, NW // 8], f8)
    whh_b = nc.dram_tensor("whh_b", [2 * H, H4 // 8], bf16)
    wT_g = nc.dram_tensor("wT_g", [NCORES * KA, NW // 8], f8,
                          addr_space="Shared")
    whh_g = nc.dram_tensor("whh_g", [NCORES * 2 * H, H4 // 8], bf16,
                           addr_space="Shared")
    GRP = [list(range(NCORES))]
    BYP = mybir.AluOpType.bypass

    ksz = [128, 128, KA - 256]  # 128/128/75 (75 = 44 word + 30 char + 1 ones)
    ko = [0, 128, 256]

    with tile.TileContext(nc) as tc:
        with tc.tile_pool(name="wp", bufs=1) as wp, \
             tc.tile_pool(name="gxp", bufs=1) as gxp, \
             tc.tile_pool(name="stp", bufs=1) as stp, \
             tc.tile_pool(name="gp", bufs=2) as gp, \
             tc.tile_pool(name="wk", bufs=3) as wk, \
             tc.tile_pool(name="pm", bufs=2, space="PSUM") as pm, \
             tc.tile_pool(name="pg", bufs=2, space="PSUM") as pg, \
             tc.tile_pool(name="pc", bufs=1, space="PSUM") as pc:

            # AllGather the weight shards (each core holds 1/8 of the gates)
            nc.sync.dma_start(wT_b[:, :], wsh[:, :])
            nc.sync.dma_start(whh_b[:, :], whh_s[:, :])
            nc.gpsimd.collective_compute("AllGather", BYP, GRP,
                                         ins=[wT_b[:, :]], outs=[wT_g[:, :]])
            nc.gpsimd.collective_compute("AllGather", BYP, GRP,
                                         ins=[whh_b[:, :]],
                                         outs=[whh_g[:, :]])

            # ---- build featT tiles on device ----
            # ft0/ft1: word-emb dims 0..255; ft2: word 256..299 | char | ones
            ft = [wp.tile([ksz[k], R], f8, name=f"ft{k}", tag=f"ft{k}")
                  for k in range(3)]
            sidx_sb = wp.tile([128, 8], i32, name="sidx_sb", tag="sidx")
            nc.sync.dma_start(sidx_sb[:, :], pcall[0:4, :].bitcast(i32))
            ident = wp.tile([128, 128], bf16, name="ident", tag="ident")
            make_identity(nc, ident[:, :])
            if bisect == "noft":
                for k in range(3):
                    nc.vector.memset(ft[k][:, :], 0.25)
            for m in ([] if bisect == "noft" else range(8)):
                g_m = gp.tile([128, WORD_E], f8, name=f"g{m}", tag="gath")
                nc.gpsimd.indirect_dma_start(
                    out=g_m[:, :],
                    out_offset=None,
                    in_=wtab[:, :],
                    in_offset=bass.IndirectOffsetOnAxis(
                        ap=sidx_sb[:, m:m + 1], axis=0),
                )
                # PE fp8 transpose has 16-bit interleave constraints; bounce
                # through bf16 instead
                g16 = gp.tile([128, WORD_E], bf16, name=f"g16_{m}",
                              tag="g16")
                nc.scalar.copy(g16[:, :], g_m[:, :])
                for k in range(3):
                    kw = ksz[k] if k < 2 else 44  # word rows in this k-tile
                    tp = pm.tile([kw, 128], bf16, tag="mm")
                    nc.tensor.transpose(tp[:, :], g16[:, ko[k]:ko[k] + kw],
                                        ident[:, :])
                    nc.scalar.copy(ft[k][0:kw, m * 128:(m + 1) * 128],
                                   tp[:, :])
            # ones row sits at partition 74; compute engines can only start
            # at partition 0/32/64/96, so memset 64:75 first and let the
            # char-feature DMA overwrite 44:74 afterwards.
            nc.vector.memset(ft[2][64:75, :], 1.0)
            nc.sync.dma_start(ft[2][44:44 + CHAR_C, :],
                              pcall[4:34, :].bitcast(f8))

            # stationary wT K-tiles (gathered weights)
            wt = []
            for k in range(3):
                t2 = wp.tile([ksz[k], NW], f8, name=f"wt{k}", tag=f"wt{k}")
                for c in range(NCORES):
                    nc.sync.dma_start(
                        t2[:, c * (NW // 8):(c + 1) * (NW // 8)],
                        wT_g[c * KA + ko[k]:c * KA + ko[k] + ksz[k], :])
                wt.append(t2)
            whh = {}
            for d in range(2):
                t0 = wp.tile([128, H4], bf16, name=f"whh{d}0", tag=f"whh{d}0")
                t1 = wp.tile([128, H4], bf16, name=f"whh{d}1", tag=f"whh{d}1")
                for c in range(NCORES):
                    base = c * 2 * H + d * H
                    nc.sync.dma_start(
                        t0[:, c * (H4 // 8):(c + 1) * (H4 // 8)],
                        whh_g[base:base + 128, :])
                    nc.sync.dma_start(
                        t1[:, c * (H4 // 8):(c + 1) * (H4 // 8)],
                        whh_g[base + 128:base + 256, :])
                whh[d] = (t0, t1)
            lint = []
            for k in range(4):
                t = wp.tile([128, NCLS], bf16, name=f"lin{k}", tag=f"lin{k}")
                nc.sync.dma_start(t[:, :], linT[k * 128:(k + 1) * 128, :])
                lint.append(t)
            linb_sb = wp.tile([1, NCLS], bf16, name="linb_sb", tag="linb")
            nc.sync.dma_start(linb_sb[:, :], linb[:, :])
            expT_sb = wp.tile([NCLS, NCLS], f32, name="expT_sb", tag="expT")
            nc.sync.dma_start(expT_sb[:, :], expT[:, :])
            cend_sb = wp.tile([NCLS, 2], f32, name="cend_sb", tag="cend")
            nc.sync.dma_start(cend_sb[:, :], cend[:, :])
            estart_sb = wp.tile([NCLS, 1], f32, name="estart_sb", tag="est")
            nc.sync.dma_start(estart_sb[:, :], estart[:, :])
            wtg8 = wp.tile([NCLS, R], f8, name="wtg8", tag="wtg8")
            nc.sync.dma_start(wtg8[:, :], pcall[34:59, :].bitcast(f8))
            ones1 = wp.tile([1, NCLS], f32, name="ones1", tag="ones1")
            nc.vector.memset(ones1[:, :], 1.0)
            onesR = wp.tile([1, R], bf16, name="onesR", tag="onesR")
            nc.vector.memset(onesR[:, :], 1.0)

            # gxT per direction: [128, S, 8, BC] fp32
            #   gxT_d[p, t, j, e] = gate (j*128+p) of dir d at time t, example e
            #   within-dir gate-slice order j: i0 i1 f0 f1 o0 o1 g0 g1
            gxT = [gxp.tile([128, S, 8, BC], f32, name=f"gxT{d}", tag=f"gxT{d}")
                   for d in range(2)]
            for d in range(2):
                for j in range(8):
                    gi = d * 8 + j
                    for rc in range(2):
                        ps = pm.tile([128, S // 2, BC], f32, tag="mm")
                        for k in range(3):
                            nc.tensor.matmul(
                                ps[:, :, :],
                                wt[k][:, gi * 128:(gi + 1) * 128],
                                ft[k][:, rc * 512:(rc + 1) * 512],
                                start=(k == 0), stop=(k == 2))
                        nc.scalar.copy(
                            gxT[d][:, rc * (S // 2):(rc + 1) * (S // 2), j, :],
                            ps[:, :, :])

            # hcat[0,1] = fwd h dims 0:128/128:256, hcat[2,3] = rev; col=t*8+e
            hcat = [stp.tile([128, R], bf16, name=f"hcat{j}", tag=f"hcat{j}")
                    for j in range(4)]
            # c state per dir: [128, 2, BC] (h dims 0:128 | 128:256)
            cst = [stp.tile([128, 2, BC], f32, name=f"c{d}", tag=f"c{d}")
                   for d in range(2)]

            from concourse.bass import ds

            def lstm_cell(d, gsl, first, hw_cols):
                """One LSTM cell update for direction d reading gates from
                gsl ([128,(1,)8,BC] pre-activation) and writing h to
                hcat[2d..2d+1][:, hw_cols]."""
                c_sb = cst[d]
                acts = wk.tile([128, 8, BC], f32, name=f"acts{d}",
                               tag=f"acts{d}")
                nc.scalar.activation(acts[:, 0:6, :], gsl[:, 0:6, :],
                                     AF.Sigmoid)
                nc.scalar.activation(acts[:, 6:8, :], gsl[:, 6:8, :], AF.Tanh)
                if first:
                    nc.vector.tensor_mul(c_sb[:, :, :], acts[:, 0:2, :],
                                         acts[:, 6:8, :])
                else:
                    fc = wk.tile([128, 2, BC], f32, name=f"fc{d}", tag=f"fc{d}")
                    nc.vector.tensor_mul(fc[:, :, :], acts[:, 2:4, :],
                                         c_sb[:, :, :])
                    nc.vector.tensor_mul(c_sb[:, :, :], acts[:, 0:2, :],
                                         acts[:, 6:8, :])
                    nc.vector.tensor_add(c_sb[:, :, :], c_sb[:, :, :],
                                         fc[:, :, :])
                th = wk.tile([128, 2, BC], f32, name=f"th{d}", tag=f"th{d}")
                nc.scalar.activation(th[:, :, :], c_sb[:, :, :], AF.Tanh)
                nc.vector.tensor_mul(hcat[2 * d][:, hw_cols],
                                     acts[:, 4, :], th[:, 0, :])
                nc.vector.tensor_mul(hcat[2 * d + 1][:, hw_cols],
                                     acts[:, 5, :], th[:, 1, :])

            # step 0 (no h feedback): gates come straight from gxT
            lstm_cell(0, gxT[0][:, 0], True, slice(0, BC))
            lstm_cell(1, gxT[1][:, S - 1], True, slice((S - 1) * BC, S * BC))

            # steps 1..S-1 as a hardware loop; sv = step index
            with tc.For_i(1, S, 1) as sv:
                for d in range(2):
                    # fwd: tt=sv prev=sv-1 ; rev: tt=S-1-sv prev=S-sv
                    if d == 0:
                        gcol = sv
                        pcol = sv * BC - BC
                        wcol = sv * BC
                    else:
                        gcol = S - 1 - sv
                        pcol = S * BC - sv * BC
                        wcol = (S - 1) * BC - sv * BC
                    ps = pg.tile([128, 8, BC], f32, name=f"rps{d}",
                                 tag=f"rps{d}")
                    w0, w1 = whh[d]
                    h0 = hcat[2 * d][:, ds(pcol, BC)]
                    h1 = hcat[2 * d + 1][:, ds(pcol, BC)]
                    for j in range(8):
                        nc.tensor.matmul(ps[:, j, :],
                                         w0[:, j * 128:(j + 1) * 128], h0,
                                         start=True, stop=False)
                        nc.tensor.matmul(ps[:, j, :],
                                         w1[:, j * 128:(j + 1) * 128], h1,
                                         start=False, stop=True)
                    g_sb = wk.tile([128, 8, BC], f32, name=f"g{d}",
                                   tag=f"g{d}")
                    nc.vector.tensor_add(g_sb[:, :, :], ps[:, :, :],
                                         gxT[d][:, ds(gcol, 1)])
                    lstm_cell(d, g_sb, False, ds(wcol, BC))

            # ---- emissions on device: emT[c, r] = (lin_w @ h)[c] + lin_b[c]
            emT = stp.tile([NCLS, R], f32, name="emT", tag="emT")
            emE = stp.tile([NCLS, R], f32, name="emE", tag="emE")
            for half in range(2):
                cols = slice(half * 512, (half + 1) * 512)
                pe = pm.tile([NCLS, 512], f32, tag="mm")
                for k in range(4):
                    nc.tensor.matmul(pe[:, :], lint[k][:, :],
                                     hcat[k][:, cols], start=(k == 0),
                                     stop=False)
                nc.tensor.matmul(pe[:, :], linb_sb[:, :], onesR[:, cols],
                                 start=False, stop=True)
                nc.scalar.copy(emT[:, cols], pe[:, :])
                nc.scalar.activation(emE[:, cols], pe[:, :], AF.Exp)

            # ---- gold-path emission sum, reduced to [1,BC] on device ----
            lzbuf = stp.tile([2, LZC], bf16, name="lzbuf", tag="lzbuf")
            wtgf = stp.tile([NCLS, R], f32, name="wtgf", tag="wtgf")
            nc.scalar.copy(wtgf[:, :], wtg8[:, :])
            nmv = stp.tile([NCLS, R], f32, name="nmv", tag="nmv")
            nc.vector.tensor_mul(nmv[:, :], emT[:, :], wtgf[:, :])
            w = R // 2
            while w >= BC:
                nc.vector.tensor_add(nmv[:, 0:w], nmv[:, 0:w],
                                     nmv[:, w:2 * w])
                w //= 2
            nmred = stp.tile([1, BC], f32, name="nmred", tag="nmred")
            nc.gpsimd.tensor_reduce(nmred[:, :], nmv[:, 0:BC],
                                    mybir.AxisListType.C,
                                    mybir.AluOpType.add)
            # bit-pack the f32 sums into bf16 cols 1024:1040 of row 0
            nc.vector.memset(lzbuf[0:2, R:R + 16], 0.0)
            nc.scalar.copy(lzbuf[0:1, R:R + 16].bitcast(f32), nmred[:, :])

            # ---- CRF forward algorithm, scaled-probability form ----
            # state: ahat[j,e] = exp(alpha_t[j,e] - off_t[e]), sum = 1
            # lzbuf row0[t] = ln s_t (per-step normalizer)
            # lzbuf row1[t] = ln sum_j v_t[j]*exp(end[j]) (logZ c. - off)
            ahat = stp.tile([NCLS, BC], f32, name="ahat", tag="ahat")

            def crf_step(v_sb, crf_ps):
                # s2[0]=sum_j v, s2[1]=sum_j v*expEnd; then renormalize
                nc.tensor.matmul(crf_ps[0:2, 16:24], cend_sb[:, :],
                                 v_sb[:, :], start=True, stop=True)
                rcp = wk.tile([1, BC], f32, name="rcp", tag="rcp")
                nc.vector.reciprocal(rcp[:, :], crf_ps[0:1, 16:24])
                nc.tensor.matmul(crf_ps[:, 8:16], ones1[:, :], rcp[:, :],
                                 start=True, stop=True)
                nc.vector.tensor_mul(ahat[:, :], v_sb[:, :],
                                     crf_ps[:, 8:16])

            # t = 0: v0 = exp(start) * exp(em_0)
            crf0 = pc.tile([NCLS, 24], f32, name="crf0", tag="crf")
            v0 = wk.tile([NCLS, BC], f32, name="v0", tag="vcrf")
            nc.vector.tensor_scalar_mul(v0[:, :], emE[:, 0:BC],
                                        estart_sb[:, 0:1])
            crf_step(v0, crf0)
            nc.scalar.activation(lzbuf[0:2, 0:BC], crf0[0:2, 16:24],
                                 AF.Ln)

            with tc.For_i(1, S, 1) as tv:
                crf_ps = pc.tile([NCLS, 24], f32, name="crfl", tag="crf")
                nc.tensor.matmul(crf_ps[:, 0:8], expT_sb[:, :],
                                 ahat[:, :], start=True, stop=True)
                v_sb = wk.tile([NCLS, BC], f32, name="v_sb", tag="vcrf")
                nc.vector.tensor_mul(v_sb[:, :], crf_ps[:, 0:8],
                                     emE[:, ds(tv * BC, BC)])
                crf_step(v_sb, crf_ps)
                nc.scalar.activation(lzbuf[0:2, ds(tv * BC, BC)],
                                     crf_ps[0:2, 16:24], AF.Ln)

            nc.sync.dma_start(lz[:, :], lzbuf[:, :])
    nc.compile()
    return nc


_NC_CACHE = {}
LAST_DEVICE_NS = [0]


def _get_runner():
    """Build the Bass module and a cached jitted shard_map executor once.

    run_bass_kernel_spmd re-creates and re-jits its closure on every call,
    which under axon costs ~100ms+ of retrace/cache-lookup per invocation.
    Here the jitted function survives across kernel() calls.
    """
    if "runner" in _NC_CACHE:
        return _NC_CACHE["runner"]
    import jax
    from jax.experimental.shard_map import shard_map
    from jax.sharding import Mesh, PartitionSpec
    from concourse import bass2jax
    import concourse.mybir as mybir

    nc = _build_nc()
    bass2jax.install_neuronx_cc_hook()
    pt = nc.partition_id_tensor
    partition_name = pt.name if pt is not None else None
    in_names, out_names, out_avals = [], [], []
    for alloc in nc.m.functions[0].allocations:
        if not isinstance(alloc, mybir.MemoryLocationSet):
            continue
        name = alloc.memorylocations[0].name
        if alloc.kind == "ExternalInput":
            if name != partition_name:
                in_names.append(name)
        elif alloc.kind == "ExternalOutput":
            shape = tuple(alloc.tensor_shape)
            dtype = mybir.dt.np(alloc.dtype)
            out_names.append(name)
            out_avals.append(jax.core.ShapedArray(shape, dtype))
    n_params = len(in_names)
    n_outs = len(out_names)
    all_in_names = tuple(in_names + out_names +
                         ([partition_name] if partition_name else []))
    donate = tuple(range(n_params, n_params + n_outs))

    def _body(*args):
        operands = list(args)
        if partition_name is not None:
            operands.append(bass2jax.partition_id_tensor())
        outs = bass2jax._bass_exec_p.bind(
            *operands,
            out_avals=tuple(out_avals),
            in_names=all_in_names,
            out_names=tuple(out_names),
            lowering_input_output_aliases=(),
            sim_require_finite=True,
            sim_require_nnan=True,
            nc=nc,
        )
        return tuple(outs)

    devices = jax.devices()[:NCORES]
    mesh = Mesh(np.asarray(devices), ("core",))
    P = PartitionSpec("core")
    # No donation: the NEFF renames outputs to output{i}, so the zero-buffer
    # operands are never read — they only exist to satisfy the bass_exec
    # operand list. Both outputs are fully written by the kernel, so fresh
    # (uninitialized) PJRT result buffers are fine, and the zeros can be
    # device-resident arrays reused across calls (donation would consume
    # them and force a re-upload every call, ~20-40ms on axon).
    del donate
    sharded = jax.jit(
        shard_map(_body, mesh=mesh, in_specs=(P,) * (n_params + n_outs),
                  out_specs=(P,) * n_outs, check_rep=False),
        keep_unused=True)
    runner = (sharded, tuple(in_names), tuple(out_names), tuple(out_avals),
              mesh)
    _NC_CACHE["runner"] = runner
    return runner


def _run_device(call_maps):
    """Run the cached NEFF; call_maps holds the per-call per-core tensors."""
    import time
    import jax
    from jax.sharding import NamedSharding, PartitionSpec

    sharded, in_names, out_names, out_avals, mesh = _get_runner()
    args = []
    for name in in_names:
        if name in _WEIGHT_NAMES:
            args.append(_NC_CACHE[("dev", name)])
        else:
            args.append(np.concatenate(
                [np.ascontiguousarray(call_maps[c][name])
                 for c in range(NCORES)], axis=0))
    if "zeros" not in _NC_CACHE:
        ns = NamedSharding(mesh, PartitionSpec("core"))
        _NC_CACHE["zeros"] = [
            jax.device_put(
                np.zeros((NCORES * a.shape[0], *a.shape[1:]), a.dtype), ns)
            for a in out_avals]
    zeros = _NC_CACHE["zeros"]

    t0 = time.time()
    outs = sharded(*args, *zeros)
    outs = [np.asarray(o) for o in outs]
    LAST_DEVICE_NS[0] = int((time.time() - t0) * 1e9)
    res = []
    for c in range(NCORES):
        res.append({name: outs[i].reshape(NCORES, *out_avals[i].shape)[c]
                    for i, name in enumerate(out_names)})
    return res


def kernel(word_table, char_table, conv_w, conv_b, w_ih_f, w_hh_f, b_f,
           w_ih_r, w_hh_r, b_r, lin_w, lin_b, start_t, end_t, trans,
           sent, word, tag, mask):
    import ml_dtypes
    bf = ml_dtypes.bfloat16
    f8 = ml_dtypes.float8_e4m3
    word_table = np.asarray(word_table, np.float32)
    char_table = np.asarray(char_table, np.float32)
    conv_w = np.asarray(conv_w, np.float32)
    conv_b = np.asarray(conv_b, np.float32)
    lin_w = np.asarray(lin_w, np.float32)
    lin_b = np.asarray(lin_b, np.float32)
    start_t = np.asarray(start_t, np.float32)
    end_t = np.asarray(end_t, np.float32)
    trans = np.asarray(trans, np.float32)
    sent_i = np.asarray(sent).astype(np.int64)
    word_i = np.asarray(word).astype(np.int64)
    tag_i = np.asarray(tag).astype(np.int64)
    mask_b = np.asarray(mask).astype(bool)

    # --- char CNN (host: tiny) ---
    ct = char_table.copy()
    ct[0] = 0.0
    cemb = ct[word_i.reshape(-1)].reshape(B * S, LW, CHAR_E)
    pad = np.zeros((B * S, LW + 2, CHAR_E), np.float32)
    pad[:, 1:LW + 1, :] = cemb
    conv = np.zeros((B * S, LW, CHAR_C), np.float32)
    for dk in range(3):
        conv += pad[:, dk:dk + LW, :] @ conv_w[:, :, dk].T
    conv += conv_b[None, None, :]
    char_feat = conv.max(axis=1).reshape(B, S, CHAR_C)  # [B,S,30]

    # --- static weights (device-resident) ---
    gp = _GATE_PERM
    wcat = np.concatenate([np.asarray(w_ih_f, np.float32)[gp],
                           np.asarray(w_ih_r, np.float32)[gp]], axis=0)
    bcat = np.concatenate([np.asarray(b_f, np.float32)[gp],
                           np.asarray(b_r, np.float32)[gp]])
    wT = np.empty((KA, NW), np.float32)
    wT[:F] = wcat.T
    wT[F] = bcat
    wT = wT.astype(f8)
    whhT_f_a = np.ascontiguousarray(
        np.asarray(w_hh_f, np.float32)[gp].T).astype(bf)
    whhT_r_a = np.ascontiguousarray(
        np.asarray(w_hh_r, np.float32)[gp].T).astype(bf)
    linT = np.ascontiguousarray(lin_w.T).astype(bf)

    if _NC_CACHE.get("wtab8_id") != id(word_table):
        _NC_CACHE["wtab8"] = word_table.astype(f8)
        _NC_CACHE["wtab8_id"] = id(word_table)
    wmap = dict(
        wtab=_NC_CACHE["wtab8"],
        expT=np.exp(trans).astype(np.float32),
        cend=np.stack([np.ones(NCLS, np.float32),
                       np.exp(end_t)], axis=1).astype(np.float32),
        estart=np.exp(start_t).astype(np.float32)[:, None],
        linb=lin_b[None, :].astype(bf),
    )
    # per-core sharded weights go in as the concat of per-core blocks
    gs, hs = NW // 8, H4 // 8
    wmap["wsh"] = np.ascontiguousarray(
        np.concatenate([wT[:, c * gs:(c + 1) * gs] for c in range(NCORES)],
                       axis=0)).reshape(NCORES * KA, gs)
    wmap["whh_s"] = np.concatenate(
        [np.concatenate([whhT_f_a[:, c * hs:(c + 1) * hs],
                         whhT_r_a[:, c * hs:(c + 1) * hs]], axis=0)
         for c in range(NCORES)], axis=0)
    wmap["linT"] = np.concatenate([linT] * NCORES, axis=0)

    _ensure_weights_pre(wmap)

    # --- per-call tensors ---
    call_maps = []
    for c in range(NCORES):
        sl = sent_i[c * BC:(c + 1) * BC].T.reshape(-1)  # r = t*BC+e
        sidx = np.ascontiguousarray(
            sl.reshape(8, 128).T).astype(np.int32)      # [p, m], r=m*128+p
        cf = char_feat[c * BC:(c + 1) * BC]             # [BC,S,30]
        charT = np.ascontiguousarray(
            cf.transpose(2, 1, 0).reshape(CHAR_C, R)).astype(f8)
        tg = tag_i[c * BC:(c + 1) * BC].T               # [S,BC]
        mk = mask_b[c * BC:(c + 1) * BC].T.astype(np.float32)
        wgt = mk.copy()
        wgt[0] = 1.0
        oh = (tg[:, :, None] == np.arange(NCLS)[None, None, :])
        Wt = (oh * wgt[:, :, None]).transpose(2, 0, 1).reshape(NCLS, R)
        pcall = np.concatenate([
            np.frombuffer(sidx.tobytes(), np.uint8).reshape(4, 1024),
            np.ascontiguousarray(charT).view(np.uint8),
            np.ascontiguousarray(Wt.astype(f8)).view(np.uint8),
        ], axis=0)
        call_maps.append(dict(pcall=pcall))
    res = _run_device(call_maps)

    # --- host: assemble NLL ---
    total = np.float64(0.0)
    for c in range(NCORES):
        lzraw = res[c]["lz"]                    # [2, R+16] bf16
        lzv = lzraw[:, :R].astype(np.float32)
        ls = lzv[0].reshape(S, BC)
        zc = lzv[1].reshape(S, BC)
        tg = tag_i[c * BC:(c + 1) * BC].T       # [S,BC]
        mk = mask_b[c * BC:(c + 1) * BC].T.astype(np.float32)
        # f32 sums bit-packed into bf16 cols R:R+16 of row 0
        emsum = np.frombuffer(
            np.ascontiguousarray(lzraw[0, R:R + 16]).tobytes(),
            dtype=np.float32).copy()            # [BC]
        tr = trans[tg[:-1], tg[1:]]             # [S-1,BC]
        score = start_t[tg[0]] + emsum + np.sum(mk[1:] * tr, axis=0)
        last = mk.sum(0).astype(np.int64) - 1
        eidx = np.arange(BC)
        score = score + end_t[tg[last, eidx]]
        cum = np.concatenate([np.zeros((1, BC), np.float64),
                              np.cumsum(ls.astype(np.float64), axis=0)],
                             axis=0)            # cum[t] = sum_{tau<t} ls
        logZ = cum[last, eidx] + zc[last, eidx]
        total += np.sum(score.astype(np.float64) - logZ)
    return np.asarray(-total, np.float32)


def _ensure_weights_pre(wmap):
    """Like _ensure_weights but wsh/whh_s/linT arrive pre-concatenated."""
    import hashlib
    import jax
    from jax.sharding import NamedSharding, PartitionSpec

    sharded, in_names, out_names, out_avals, mesh = _get_runner()
    ns = NamedSharding(mesh, PartitionSpec("core"))
    h = hashlib.blake2b(digest_size=16)
    for name in _WEIGHT_NAMES:
        a = np.ascontiguousarray(wmap[name])
        h.update(repr(a.shape).encode())
        b = a.view(np.uint8).reshape(-1)
        if b.nbytes > (1 << 20):
            h.update(bytes(b[:: max(1, b.nbytes // (1 << 18))]))
            h.update(bytes(b[-4096:]))
        else:
            h.update(b.tobytes())
    fp = h.digest()
    if _NC_CACHE.get("wfp") != fp:
        pre = ("wsh", "whh_s", "linT")
        for name in _WEIGHT_NAMES:
            a = np.ascontiguousarray(wmap[name])
            if name in pre:
                g = a
            else:
                g = np.concatenate([a] * NCORES, axis=0)
            _NC_CACHE[("dev", name)] = jax.device_put(g, ns)
        _NC_CACHE["wfp"] = fp
